# revision 7
# baseline (speedup 1.0000x reference)
"""Trainium2 Bass kernel for the ChaosModulator recurrence (v4).

Math (per (b,c) sequence, t = 0..4095), tracking v = 0.25*s (s-form state):
    e_t = x_t - 7*v_{t-1}^2
    h_t = tanh(0.5*e_t + 0.875)
    v_t = 0.5*v_{t-1} + 0.25*h_t
    u_t = 0.5*x_t + v_t

The map contracts ~0.5/step, so each 16-step output span is computed by an
independent chain warmed up W=8 steps from v=0 (rel-l2 ~2e-3 incl. bf16
state/output rounding, vs the 2e-2 budget).  4096 steps -> 256 sub-chains
per sequence, processed as 1024-wide SIMD columns (16 groups x 64
sub-chains) over 4 time-batches of 1024 steps, two batches interleaved in
emission so ACT(tanh) and DVE(two fused custom ops) pipeline.

Per-core layout (2048 seqs = 128 partitions x 16 groups):
  Xb[p, g, 1040] f32   row-major x, one big-descriptor SWDGE DMA per batch
  Ub[p, g, 1024] bf16  v-state written at its final u position (strided
                       1-elem writes; ~+0.8ns/elem on DVE), turned into u by
                       one bulk fused op, then cast-DMA'd out contiguously.
Round ops are [128 x 1024]: ACT tanh + custom DVE LINCOMB (c0*v+c1*h) +
custom DVE CHAOS_E (x - 7*v^2); 24 rounds/batch.
"""

import numpy as np

import concourse.bacc as bacc
import concourse.dve_ops as dve_ops
import concourse.mybir as mybir
from concourse.bass_utils import run_bass_kernel_spmd
from concourse.dve_spec import C0, C1, Spec, Src0, Src1, _has_src1, lower, sq
from concourse.dve_uop import DveOpSpec
from concourse.tile import TileContext

F32 = mybir.dt.float32
BF16 = mybir.dt.bfloat16

P = 128             # SBUF partitions
G = 16              # sequence groups per core (2048 = G*P)
T = 4096
S = 16              # sub-chain output steps
W = 8               # warmup steps
CH = 1024           # time-steps per batch
NBATCH = T // CH    # 4
NSUB = CH // S      # 64 sub-chains per (g, batch)
C = G * NSUB        # 1024 chain columns per batch
XW = CH + S         # x tile width (c-viewable: 65*16); only CH+W loaded
NSEQ = P * G
NCORES = 8

_MULT = mybir.AluOpType.mult
_ADD = mybir.AluOpType.add


def _register_custom_ops():
    """Register the two fused DVE ops (idempotent)."""
    if "CHAOS_E" in dve_ops._SUB_OPCODE_FOR_NAME:
        by = {op.name: op for op in dve_ops.OPS}
        return by["CHAOS_E"], by["CHAOS_U"]

    spec_e = Spec(
        body=Src1 - C0 * sq(Src0),
        reference=lambda in0, in1, s0: in1 - s0 * in0 * in0,
    )
    spec_u = Spec(
        body=C0 * Src0 + C1 * Src1,
        reference=lambda in0, in1, s0, s1: s0 * in0 + s1 * in1,
    )
    ops = []
    for name, spec in (("CHAOS_E", spec_e), ("CHAOS_U", spec_u)):
        op = dve_ops.DveOp(name, spec, subdim=False, uops_sha={})
        dve_ops.OPS.append(op)
        dve_ops.CUSTOM_DVE_SPECS[name] = spec
        dve_ops._SUB_OPCODE_FOR_NAME[name] = (
            dve_ops._CUSTOM_DVE_ROW_BASE + len(dve_ops.OPS) - 1
        )
        for ver in ("v3", "v4"):
            try:
                s = DveOpSpec(
                    name=name,
                    opcode=dve_ops.get_dve_sub_opcode(name),
                    uops=lower(spec, ver=ver),
                    rd1_en=_has_src1(spec),
                )
                op.uops_sha[ver] = s.sha(ver)
            except Exception:
                pass
        ops.append(op)
    return ops


def _build_nc():
    CHAOS_E, CHAOS_U = _register_custom_ops()
    nc = bacc.Bacc("TRN2", target_bir_lowering=False, debug=False)

    x = nc.dram_tensor("x", [NSEQ, T], F32, kind="ExternalInput")
    z0 = nc.dram_tensor("z0", [NSEQ], F32, kind="ExternalInput")
    u = nc.dram_tensor("u", [NSEQ, T], F32, kind="ExternalOutput")

    xr = x[:, :].rearrange("(g p) t -> p g t", p=P)    # [128, 16, 4096]
    ur = u[:, :].rearrange("(g p) t -> p g t", p=P)
    z0r = z0[:].rearrange("(g p) -> p g", p=P)         # [128, 16]

    with TileContext(nc) as tc:
        with (
            tc.tile_pool(name="xp", bufs=2) as xp,
            tc.tile_pool(name="up", bufs=1) as up,
            tc.tile_pool(name="sp", bufs=2) as sp,
            tc.tile_pool(name="cp", bufs=1) as cp,
        ):
            bias_t = cp.tile([P, 1], F32, name="bias")
            nc.vector.memset(bias_t[:, :], 0.875)
            z0_t = cp.tile([P, G], F32, name="z0t")
            nc.sync.dma_start(out=z0_t[:, :], in_=z0r)
            # v_init = 0.25*(4*z0-2) = z0 - 0.5
            z0p = cp.tile([P, G], F32, name="z0p")
            nc.scalar.activation(
                out=z0p[:, :], in_=z0_t[:, :],
                func=mybir.ActivationFunctionType.Copy, bias=-0.5)

            tiles = {}
            for bt in range(NBATCH):
                Xb = xp.tile([P, G, XW], F32, name=f"X{bt}", tag="X")
                if bt == 0:
                    nc.vector.memset(Xb[:, :, 0:W], 0.0)
                    nc.gpsimd.dma_start(
                        out=Xb[:, :, W:W + CH], in_=xr[:, :, 0:CH])
                else:
                    t0 = bt * CH - W
                    nc.gpsimd.dma_start(
                        out=Xb[:, :, 0:W + CH], in_=xr[:, :, t0:t0 + W + CH])
                Ub = up.tile([P, G, CH], BF16, name=f"U{bt}", tag="U")
                h1 = sp.tile([P, G, NSUB], BF16, name=f"h{bt}", tag="h")
                e1 = sp.tile([P, G, NSUB], BF16, name=f"e{bt}", tag="e")
                vw = sp.tile([P, G, NSUB], F32, name=f"v{bt}", tag="v")
                tiles[bt] = (Xb, Ub, h1, e1, vw)

            def round_ops(bt, k):
                Xb, Ub, h1, e1, vw = tiles[bt]
                Xv = Xb.rearrange("p g (c s) -> p g c s", s=S)
                Uv = Ub.rearrange("p g (c s) -> p g c s", s=S)
                h = h1[:, :, :]
                # h_k = tanh(0.5*e_k + 0.875); e_0 = x_0 read straight from X
                src = Xv[:, :, 0:NSUB, 0] if k == 0 else e1[:, :, :]
                nc.scalar.activation(
                    out=h, in_=src,
                    func=mybir.ActivationFunctionType.Tanh,
                    bias=bias_t[:, :], scale=0.5)
                # v_k = 0.5*v_{k-1} + 0.25*h_k   (v_{-1} = 0)
                if k == 0:
                    v_prev, s0, s1 = h, 0.125, 0.125
                else:
                    v_prev = (vw[:, :, :] if k <= W
                              else Uv[:, :, :, k - 1 - W])
                    s0, s1 = 0.5, 0.25
                v_out = vw[:, :, :] if k < W else Uv[:, :, :, k - W]
                nc.vector._custom_dve(
                    CHAOS_U, out=v_out, in0=v_prev, in1=h, s0=s0, s1=s1)
                if bt == 0 and k == W - 1:
                    # replace warmup state of the t=0 sub-chain with true z0
                    nc.vector.tensor_copy(out=vw[:, :, 0], in_=z0p[:, :])
                    v_out = vw[:, :, :]
                # e_{k+1} = x_{k+1} - 7*v_k^2
                if k < W + S - 1:
                    kk = k + 1
                    xin = (Xv[:, :, 0:NSUB, kk] if kk < S
                           else Xv[:, :, 1:NSUB + 1, kk - S])
                    nc.vector._custom_dve(
                        CHAOS_E, out=e1[:, :, :], in0=v_out, in1=xin, s0=7.0)

            def tail_ops(bt):
                Xb, Ub, h1, e1, vw = tiles[bt]
                # u = 1.0*v + 0.5*x  (in place over U), then cast-DMA out
                nc.vector._custom_dve(
                    CHAOS_U, out=Ub[:, :, :], in0=Ub[:, :, :],
                    in1=Xb[:, :, W:W + CH], s0=1.0, s1=0.5)
                nc.gpsimd.dma_start(
                    out=ur[:, :, bt * CH:(bt + 1) * CH], in_=Ub[:, :, :])

            for pair in range(NBATCH // 2):
                b0, b1 = 2 * pair, 2 * pair + 1
                for k in range(W + S):
                    round_ops(b0, k)
                    round_ops(b1, k)
                tail_ops(b0)
                tail_ops(b1)

    nc.compile()
    return nc


_NC = None


def _get_nc():
    global _NC
    if _NC is None:
        _NC = _build_nc()
    return _NC


def kernel(x: np.ndarray, z0: np.ndarray) -> np.ndarray:
    x = np.ascontiguousarray(x, dtype=np.float32)      # (32, 512, 4096)
    z0 = np.ascontiguousarray(z0, dtype=np.float32)    # (32, 512)
    nc = _get_nc()

    in_maps = []
    for i in range(NCORES):
        xs = np.ascontiguousarray(x[4 * i:4 * (i + 1)].reshape(NSEQ, T))
        zs = np.ascontiguousarray(z0[4 * i:4 * (i + 1)].reshape(NSEQ))
        in_maps.append({"x": xs, "z0": zs})

    res = run_bass_kernel_spmd(nc, in_maps, core_ids=list(range(NCORES)))
    out = np.empty((32, 512, T), np.float32)
    for i in range(NCORES):
        out[4 * i:4 * (i + 1)] = res.results[i]["u"].reshape(4, 512, T)
    return out


# revision 8
# speedup vs baseline: 1.5578x; 1.5578x over previous
"""Trainium2 Bass kernel for the ChaosModulator recurrence (v5).

Math (per (b,c) sequence, t = 0..4095), tracking v = 0.25*s (s-form state):
    e_t = x_t - 7*v_{t-1}^2
    h_t = tanh(0.5*e_t + 0.875)
    v_t = 0.5*v_{t-1} + 0.25*h_t
    u_t = 0.5*x_t + v_t

The map contracts ~0.5/step, so each 16-step output span is computed by an
independent chain warmed up W=6 steps from v=0 (rel-l2 ~2.3e-3 incl. bf16
rounding, vs the 2e-2 budget).  4096 steps -> 256 sub-chains per sequence,
processed as 1024-wide SIMD columns (16 groups x 64 sub-chains) over 4
time-batches of 1024 steps, two batches interleaved in emission so
ACT(tanh) and DVE(three fused custom ops) pipeline.

Per-core layout (2048 seqs = 128 partitions x 16 groups):
  Xb[p, g, 1040] bf16  row-major x (one big-descriptor SWDGE cast-DMA/batch)
  Ub[p, g, 1024] bf16  u written directly at its final position by a
                       per-round fused op (double-buffered so the two
                       interleaved batches never serialize on it), then
                       cast-DMA'd out contiguously.
v stays f32 in a contiguous in-place tile (no strided state reads); the
only strided APs are the inherent x reads and u writes (~+0.7ns/elem).
Round ops are [128 x 1024]: ACT tanh + custom DVE LINCOMB (c0*v+c1*h) +
custom DVE CHAOS_E (x - 7*v^2) + custom DVE UOP (0.5*x + v); 22
rounds/batch, ~490 instructions total.
"""

import numpy as np

import concourse.bacc as bacc
import concourse.dve_ops as dve_ops
import concourse.mybir as mybir
from concourse.bass_utils import run_bass_kernel_spmd
from concourse.dve_spec import C0, C1, Spec, Src0, Src1, _has_src1, lower, sq
from concourse.dve_uop import DveOpSpec
from concourse.tile import TileContext

F32 = mybir.dt.float32
BF16 = mybir.dt.bfloat16

P = 128             # SBUF partitions
G = 16              # sequence groups per core (2048 = G*P)
T = 4096
S = 16              # sub-chain output steps
W = 6               # warmup steps
CH = 1024           # time-steps per batch
NBATCH = T // CH    # 4
NSUB = CH // S      # 64 sub-chains per (g, batch)
C = G * NSUB        # 1024 chain columns per batch
XW = CH + S         # x tile width (c-viewable: 65*16); only CH+W loaded
NSEQ = P * G
NCORES = 8

_MULT = mybir.AluOpType.mult
_ADD = mybir.AluOpType.add


def _register_custom_ops():
    """Register the two fused DVE ops (idempotent)."""
    if "CHAOS_E" in dve_ops._SUB_OPCODE_FOR_NAME:
        by = {op.name: op for op in dve_ops.OPS}
        return by["CHAOS_E"], by["CHAOS_U"]

    spec_e = Spec(
        body=Src1 - C0 * sq(Src0),
        reference=lambda in0, in1, s0: in1 - s0 * in0 * in0,
    )
    spec_u = Spec(
        body=C0 * Src0 + C1 * Src1,
        reference=lambda in0, in1, s0, s1: s0 * in0 + s1 * in1,
    )
    ops = []
    for name, spec in (("CHAOS_E", spec_e), ("CHAOS_U", spec_u)):
        op = dve_ops.DveOp(name, spec, subdim=False, uops_sha={})
        dve_ops.OPS.append(op)
        dve_ops.CUSTOM_DVE_SPECS[name] = spec
        dve_ops._SUB_OPCODE_FOR_NAME[name] = (
            dve_ops._CUSTOM_DVE_ROW_BASE + len(dve_ops.OPS) - 1
        )
        for ver in ("v3", "v4"):
            try:
                s = DveOpSpec(
                    name=name,
                    opcode=dve_ops.get_dve_sub_opcode(name),
                    uops=lower(spec, ver=ver),
                    rd1_en=_has_src1(spec),
                )
                op.uops_sha[ver] = s.sha(ver)
            except Exception:
                pass
        ops.append(op)
    return ops


def _build_nc():
    CHAOS_E, CHAOS_U = _register_custom_ops()
    nc = bacc.Bacc("TRN2", target_bir_lowering=False, debug=False)

    x = nc.dram_tensor("x", [NSEQ, T], F32, kind="ExternalInput")
    z0 = nc.dram_tensor("z0", [NSEQ], F32, kind="ExternalInput")
    u = nc.dram_tensor("u", [NSEQ, T], F32, kind="ExternalOutput")

    xr = x[:, :].rearrange("(g p) t -> p g t", p=P)    # [128, 16, 4096]
    ur = u[:, :].rearrange("(g p) t -> p g t", p=P)
    z0r = z0[:].rearrange("(g p) -> p g", p=P)         # [128, 16]

    with TileContext(nc) as tc:
        with (
            tc.tile_pool(name="xp", bufs=2) as xp,
            tc.tile_pool(name="up", bufs=2) as up,
            tc.tile_pool(name="sp", bufs=2) as sp,
            tc.tile_pool(name="cp", bufs=1) as cp,
        ):
            bias_t = cp.tile([P, 1], F32, name="bias")
            nc.vector.memset(bias_t[:, :], 0.875)
            z0_t = cp.tile([P, G], F32, name="z0t")
            nc.sync.dma_start(out=z0_t[:, :], in_=z0r)
            # v_init = 0.25*(4*z0-2) = z0 - 0.5
            z0p = cp.tile([P, G], F32, name="z0p")
            nc.scalar.activation(
                out=z0p[:, :], in_=z0_t[:, :],
                func=mybir.ActivationFunctionType.Copy, bias=-0.5)

            tiles = {}
            for bt in range(NBATCH):
                Xb = xp.tile([P, G, XW], BF16, name=f"X{bt}", tag="X")
                if bt == 0:
                    nc.vector.memset(Xb[:, :, 0:W], 0.0)
                    nc.gpsimd.dma_start(
                        out=Xb[:, :, W:W + CH], in_=xr[:, :, 0:CH])
                else:
                    t0 = bt * CH - W
                    nc.gpsimd.dma_start(
                        out=Xb[:, :, 0:W + CH], in_=xr[:, :, t0:t0 + W + CH])
                Ub = up.tile([P, G, CH], BF16, name=f"U{bt}", tag="U")
                h1 = sp.tile([P, G, NSUB], BF16, name=f"h{bt}", tag="h")
                e1 = sp.tile([P, G, NSUB], BF16, name=f"e{bt}", tag="e")
                vw = sp.tile([P, G, NSUB], F32, name=f"v{bt}", tag="v")
                tiles[bt] = (Xb, Ub, h1, e1, vw)

            def round_ops(bt, k):
                Xb, Ub, h1, e1, vw = tiles[bt]
                Xv = Xb.rearrange("p g (c s) -> p g c s", s=S)
                Uv = Ub.rearrange("p g (c s) -> p g c s", s=S)
                h = h1[:, :, :]
                # h_k = tanh(0.5*e_k + 0.875); e_0 = x_0 read straight from X
                src = Xv[:, :, 0:NSUB, 0] if k == 0 else e1[:, :, :]
                nc.scalar.activation(
                    out=h, in_=src,
                    func=mybir.ActivationFunctionType.Tanh,
                    bias=bias_t[:, :], scale=0.5)
                # v_k = 0.5*v_{k-1} + 0.25*h_k  (in place; v_{-1} = 0)
                if k == 0:
                    nc.vector._custom_dve(
                        CHAOS_U, out=vw[:, :, :], in0=h, in1=h,
                        s0=0.125, s1=0.125)
                else:
                    nc.vector._custom_dve(
                        CHAOS_U, out=vw[:, :, :], in0=vw[:, :, :], in1=h,
                        s0=0.5, s1=0.25)
                if bt == 0 and k == W - 1:
                    # replace warmup state of the t=0 sub-chain with true z0
                    nc.vector.tensor_copy(out=vw[:, :, 0], in_=z0p[:, :])
                # e_{k+1} = x_{k+1} - 7*v_k^2
                if k < W + S - 1:
                    kk = k + 1
                    xin = (Xv[:, :, 0:NSUB, kk] if kk < S
                           else Xv[:, :, 1:NSUB + 1, kk - S])
                    nc.vector._custom_dve(
                        CHAOS_E, out=e1[:, :, :], in0=vw[:, :, :],
                        in1=xin, s0=7.0)
                if k >= W:
                    # u_j = 0.5*x_j + v_j  directly into its output slot
                    j = k - W
                    xu = (Xv[:, :, 0:NSUB, k] if k < S
                          else Xv[:, :, 1:NSUB + 1, k - S])
                    nc.vector._custom_dve(
                        CHAOS_U, out=Uv[:, :, :, j], in0=xu,
                        in1=vw[:, :, :], s0=0.5, s1=1.0)

            def tail_ops(bt):
                Xb, Ub, h1, e1, vw = tiles[bt]
                nc.gpsimd.dma_start(
                    out=ur[:, :, bt * CH:(bt + 1) * CH], in_=Ub[:, :, :])

            for pair in range(NBATCH // 2):
                b0, b1 = 2 * pair, 2 * pair + 1
                for k in range(W + S):
                    round_ops(b0, k)
                    round_ops(b1, k)
                tail_ops(b0)
                tail_ops(b1)

    nc.compile()
    return nc


_NC = None


def _get_nc():
    global _NC
    if _NC is None:
        _NC = _build_nc()
    return _NC


def kernel(x: np.ndarray, z0: np.ndarray) -> np.ndarray:
    x = np.ascontiguousarray(x, dtype=np.float32)      # (32, 512, 4096)
    z0 = np.ascontiguousarray(z0, dtype=np.float32)    # (32, 512)
    nc = _get_nc()

    in_maps = []
    for i in range(NCORES):
        xs = np.ascontiguousarray(x[4 * i:4 * (i + 1)].reshape(NSEQ, T))
        zs = np.ascontiguousarray(z0[4 * i:4 * (i + 1)].reshape(NSEQ))
        in_maps.append({"x": xs, "z0": zs})

    res = run_bass_kernel_spmd(nc, in_maps, core_ids=list(range(NCORES)))
    out = np.empty((32, 512, T), np.float32)
    for i in range(NCORES):
        out[4 * i:4 * (i + 1)] = res.results[i]["u"].reshape(4, 512, T)
    return out
